# revision 10
# baseline (speedup 1.0000x reference)
"""Trainium2 Bass kernel for nn_CrossPairMemory.

Sharding: data-parallel over batch across 8 NeuronCores (512 rows each).

Key algebraic restructuring (host-side, exact):
  h_pre = concat(pair_corr, macro_corr) @ w1
        = attn_p @ (vP @ W1p) + attn_m @ (vM @ W1m)
  so the 7168-deep fusion contraction collapses to two 64-deep matmuls
  against precomputed [64, 3584] tables.  Likewise the per-pair output
  path folds w2 into pair_w:
  out_pre^T[m] = pw0[m]^T ps[m]^T + sum_k Wc[m,k]^T h2[k] + c[m] 1^T
  with Wc[m] = W2[:, m-block] @ pw1[m] and c[m] = b2[m-block] @ pw1[m]
  + pair_b[m], merging the second fusion Linear and the per-pair Linear
  into one accumulation per pair with batch on the free axis.  Both
  LayerNorms are column-stat normalizations via ones-matmuls + rank-1
  broadcasts on the PE; gains are per-partition stt scalars.

Engine balance: row math on GPSIMD, normalize stt alternates
vector/GPSIMD, copies on scalar, reciprocal via the fast custom-DVE
approximation.  Dummy matmuls keep the PE HAM clock-gate warm through
the DMA-bound head and the LayerNorm barrier.
"""

import sys

for _p in ("/opt/trn_rl_repo",):
    if _p not in sys.path:
        sys.path.insert(0, _p)

import numpy as np
import ml_dtypes

import concourse.bass as bass
import concourse.tile as tile
from concourse import bacc, mybir
from concourse import bass_utils

BF = ml_dtypes.bfloat16
dt = mybir.dt
AF = mybir.ActivationFunctionType
ALU = mybir.AluOpType

NCORES = 8
B, P, PD, MD, S = 4096, 28, 128, 256, 64
D = P * PD            # 3584
Bc = B // NCORES      # 512 batch rows per core
EPS = 1e-5


def _build(has_c, has_bl2, has_g2):
    nc = bacc.Bacc(
        "TRN2", target_bir_lowering=False, debug=False, num_devices=NCORES
    )

    def din(name, shape, dty):
        return nc.dram_tensor(name, list(shape), dty, kind="ExternalInput").ap()

    psT = din("psT", (P, PD, Bc), dt.bfloat16)      # pair_states^T per pair
    msT = din("msT", (MD, Bc), dt.bfloat16)         # macro_state^T
    kP = din("kP", (PD, S), dt.bfloat16)            # pair keys^T, pre-scaled
    kM = din("kM", (MD, S), dt.bfloat16)            # macro keys^T, pre-scaled
    vwp = din("vwp", (S, D), dt.bfloat16)           # vP @ W1p
    vwm = din("vwm", (S, D), dt.bfloat16)           # vM @ W1m
    vws = din("vws", (S, 2), dt.bfloat16)           # rowsums for stat_h fold
    b1t = din("b1t", (PD, P), dt.float32)
    g1t = din("g1t", (PD, P), dt.float32)
    be1t = din("be1t", (PD, P), dt.float32)
    wcr = din("wcr", (P, PD, P, PD), dt.bfloat16)   # [m, kp, kt, e]
    pw0r = din("pw0r", (PD, P, PD), dt.bfloat16)    # [d, m, e]
    cbr = din("cbr", (1, P, PD), dt.bfloat16)       # folded bias rows
    g2t = din("g2t", (PD, P), dt.float32)          # pair_ln_g^T [e, m]
    bl2t = din("bl2t", (PD, P), dt.float32)         # pair_ln_b^T [e, m]
    out = nc.dram_tensor(
        "out", [P, PD, Bc], dt.bfloat16, kind="ExternalOutput"
    ).ap()

    with tile.TileContext(nc) as tc:
        with (
            tc.tile_pool(name="const", bufs=1) as const,
            tc.tile_pool(name="res", bufs=1) as res,
            tc.tile_pool(name="psJ", bufs=1, space="PSUM") as psj,
        ):
            # -------- constants + high-priority DMAs (macro path first) ---
            kM0 = const.tile([PD, S], dt.bfloat16, tag="kM0", name="kM0")
            nc.sync.dma_start(kM0, kM[0:PD])
            kM1 = const.tile([PD, S], dt.bfloat16, tag="kM1", name="kM1")
            nc.sync.dma_start(kM1, kM[PD:MD])
            ms0 = const.tile([PD, Bc], dt.bfloat16, tag="ms0", name="ms0")
            nc.sync.dma_start(ms0, msT[0:PD])
            ms1 = const.tile([PD, Bc], dt.bfloat16, tag="ms1", name="ms1")
            nc.sync.dma_start(ms1, msT[PD:MD])
            kP_sb = const.tile([PD, S], dt.bfloat16, tag="kP", name="kP")
            nc.sync.dma_start(kP_sb, kP)

            G = P // 4
            psT_q = []
            for i in range(4):
                t = res.tile([PD, G, Bc], dt.bfloat16, tag=f"psTq{i}", name=f"psTq{i}")
                eng = nc.sync if i % 2 == 0 else nc.scalar
                eng.dma_start(
                    t,
                    bass.AP(tensor=psT.tensor, offset=psT.offset + i * G * PD * Bc,
                            ap=[[Bc, PD], [PD * Bc, G], [1, Bc]]),
                )
                psT_q.append(t)
            psT_sb = [psT_q[p // G][:, p % G, :] for p in range(P)]

            vwp_sb = res.tile([S, D], dt.bfloat16, tag="vwp", name="vwp")
            nc.scalar.dma_start(vwp_sb, vwp)
            vwm_sb = res.tile([S, D], dt.bfloat16, tag="vwm", name="vwm")
            nc.scalar.dma_start(vwm_sb, vwm)
            vws_sb = const.tile([S, 2], dt.bfloat16, tag="vws", name="vws")
            nc.scalar.dma_start(vws_sb, vws)

            lnc = {}
            for nm, src, dty in (("b1", b1t, dt.float32), ("g1", g1t, dt.float32),
                                 ("be1", be1t, dt.float32), ("g2", g2t, dt.float32),
                                 ("bl2", bl2t, dt.float32)):
                t = const.tile([PD, P], dty, tag=f"lnc_{nm}", name=f"lnc_{nm}")
                nc.sync.dma_start(t, src)
                lnc[nm] = t
            pw0_sb = const.tile([PD, P, PD], dt.bfloat16, tag="pw0", name="pw0")
            nc.sync.dma_start(pw0_sb, pw0r)
            c_sb = const.tile([1, P, PD], dt.bfloat16, tag="cb", name="cb")
            if has_c:
                nc.sync.dma_start(c_sb, cbr)

            ones_col = const.tile([PD, 1], dt.bfloat16, tag="ones_col", name="ones_col")
            nc.vector.memset(ones_col, 1.0)
            ones_row_b = const.tile([1, PD], dt.bfloat16, tag="ones_row_b", name="ones_row_b")
            nc.vector.memset(ones_row_b, 1.0)
            ones_bc = const.tile([1, Bc], dt.bfloat16, tag="ones_bc", name="ones_bc")
            nc.vector.memset(ones_bc, 1.0)
            eps1 = const.tile([1, 1], dt.float32, tag="eps1", name="eps1")
            nc.vector.memset(eps1, EPS)

            # junk psum target for PE HAM-warming dummy matmuls
            junk = psj.tile([S, Bc], dt.float32, tag="junk", name="junk")

            def dummy_mm():
                nc.tensor.matmul(junk, kP_sb, ms0, start=True, stop=True,
                                 skip_group_check=True)

            hbuf = [
                res.tile([PD, Bc], dt.bfloat16, tag=f"hb{n}", name=f"hb{n}")
                for n in range(P)
            ]
            t1buf = [
                res.tile([PD, Bc], dt.bfloat16, tag=f"t1b{n}", name=f"t1b{n}")
                for n in range(P)
            ]
            ab = {}

            # ---------------- stage A: attention weights ----------------
            with (
                tc.tile_pool(name="stA", bufs=1) as pa,
                tc.tile_pool(name="psA", bufs=2, space="PSUM") as ppa,
            ):
                def softmax_read(which):
                    sp = ppa.tile([S, Bc], dt.float32, tag="sp", name="sp")
                    if which == "pair":
                        for p in range(P):
                            dummy_mm()
                            if p < 14:
                                dummy_mm()
                            nc.tensor.matmul(
                                sp, kP_sb, psT_sb[p],
                                start=(p == 0), stop=(p == P - 1),
                                skip_group_check=True,
                            )
                    else:
                        nc.tensor.matmul(sp, kM0, ms0, start=True, stop=False)
                        nc.tensor.matmul(sp, kM1, ms1, start=False, stop=True)
                    # scores are O(0.3): exp without max-subtraction is safe
                    eb = pa.tile([S, Bc], dt.bfloat16, tag=f"eb_{which}", name=f"eb_{which}")
                    nc.scalar.activation(eb, sp, AF.Exp)
                    den = ppa.tile([1, Bc], dt.float32, tag="den", name="den")
                    nc.tensor.matmul(den, ones_col[0:S, :], eb, start=True, stop=True,
                                     skip_group_check=True)
                    rr = pa.tile([1, Bc], dt.float32, tag=f"rr_{which}", name=f"rr_{which}")
                    nc.vector.reciprocal_approx_fast(rr, den)
                    rrh = pa.tile([1, Bc], dt.bfloat16, tag=f"rrh_{which}", name=f"rrh_{which}")
                    nc.scalar.activation(rrh, rr, AF.Copy)
                    rbc = ppa.tile([S, Bc], dt.float32, tag="rbc", name="rbc")
                    nc.tensor.matmul(
                        rbc, ones_row_b[:, 0:S], rrh, start=True, stop=True,
                        skip_group_check=True,
                    )
                    t = res.tile([S, Bc], dt.bfloat16, tag=f"ab_{which}", name=f"ab_{which}")
                    nc.vector.tensor_mul(t, eb, rbc)
                    ab[which] = t

                softmax_read("macro")
                softmax_read("pair")

            # ------------- stage B: folded fusion Linear1 + stats --------
            with tc.tile_pool(name="stat", bufs=1, space="PSUM") as pst:
                stat_h = pst.tile([1, Bc], dt.float32, tag="stat_h", name="stat_h")
                stat_q = pst.tile([1, Bc], dt.float32, tag="stat_q", name="stat_q")

                with (
                    tc.tile_pool(name="psB", bufs=2, space="PSUM") as ppm,
                    tc.tile_pool(name="sqp", bufs=2) as psq,
                ):
                    sq_t = [None] * P

                    # stat_h folds through the linear algebra: softmax rows
                    # sum to 1, so sum_f h = ab_p @ rowsum(VWp + b1) +
                    # ab_m @ rowsum(VWm)  (host-folded into vws)
                    nc.tensor.matmul(
                        stat_h, vws_sb[:, 0:1], ab["pair"],
                        start=True, stop=False, skip_group_check=True,
                    )
                    nc.tensor.matmul(
                        stat_h, vws_sb[:, 1:2], ab["macro"],
                        start=False, stop=True, skip_group_check=True,
                    )
                    # mean side of LayerNorm1 is ready now — broadcast it
                    # early so stage B can also fold in the mean-subtract
                    mu_h = const.tile([1, Bc], dt.bfloat16, tag="muh", name="muh")
                    nc.scalar.activation(mu_h, stat_h, AF.Copy, scale=1.0 / D)
                    mu_bc0 = ppm.tile([PD, Bc], dt.float32, tag="pm", name="pm")
                    nc.tensor.matmul(mu_bc0, ones_row_b, mu_h, start=True, stop=True,
                                     skip_group_check=True)
                    mu_b = const.tile([PD, Bc], dt.bfloat16, tag="mu_b", name="mu_b")
                    nc.scalar.activation(mu_b, mu_bc0, AF.Copy)

                    def stats_for(n):
                        nc.tensor.matmul(
                            stat_q, ones_col, sq_t[n],
                            start=(n == 0), stop=(n == P - 1),
                            skip_group_check=True,
                        )

                    for n in range(P):
                        pm = ppm.tile([PD, Bc], dt.float32, tag="pm", name="pm")
                        ns = slice(n * PD, (n + 1) * PD)
                        nc.tensor.matmul(
                            pm, vwp_sb[:, ns], ab["pair"], start=True, stop=False
                        )
                        nc.tensor.matmul(
                            pm, vwm_sb[:, ns], ab["macro"], start=False, stop=True
                        )
                        # lag the stats matmuls two iterations so the PE
                        # never waits on the copy/square of the current tile
                        if n > 1:
                            stats_for(n - 2)
                        nc.scalar.activation(hbuf[n], pm, AF.Copy)
                        sq = psq.tile([PD, Bc], dt.bfloat16, tag="sq", name="sq")
                        nc.vector.scalar_tensor_tensor(
                            sq, hbuf[n], 1.0, hbuf[n],
                            op0=ALU.mult, op1=ALU.mult,
                        )
                        nc.vector.scalar_tensor_tensor(
                            t1buf[n], hbuf[n], 1.0, mu_b,
                            op0=ALU.mult, op1=ALU.subtract,
                        )
                        sq_t[n] = sq
                    stats_for(P - 2)
                    stats_for(P - 1)
                    for _ in range(16):
                        dummy_mm()

                # ---------------- LayerNorm1 + gelu ----------------------
                with tc.tile_pool(name="lnrow", bufs=1) as plr:
                    m2_row = plr.tile([1, Bc], dt.float32, tag="m2", name="m2")
                    nc.scalar.activation(m2_row, stat_q, AF.Copy, scale=1.0 / D)
                    nva = plr.tile([1, Bc], dt.float32, tag="va", name="va")
                    nc.vector.scalar_tensor_tensor(
                        nva, mu_h, 1.0, mu_h, op0=ALU.mult, op1=ALU.mult,
                    )
                    nc.vector.scalar_tensor_tensor(
                        nva, nva, 1.0, m2_row, op0=ALU.mult, op1=ALU.subtract,
                    )
                    # nva = mu^2 - m2 = -var; sqrt(nva * -1 + eps) = sd
                    sd_row = plr.tile([1, Bc], dt.float32, tag="sd", name="sd")
                    nc.scalar.activation(
                        sd_row, nva, AF.Sqrt, bias=eps1, scale=-1.0
                    )
                    rstd_row = plr.tile([1, Bc], dt.float32, tag="rs", name="rs")
                    nc.vector.reciprocal_approx_fast(rstd_row, sd_row)
                    rstd_h = plr.tile([1, Bc], dt.bfloat16, tag="rsh", name="rsh")
                    nc.scalar.activation(rstd_h, rstd_row, AF.Copy)

            # stat psum closed; broadcast the rstd row
            with tc.tile_pool(name="lnb", bufs=1) as plb:
                with tc.tile_pool(name="psBC", bufs=1, space="PSUM") as ppbc:
                    rs_bc = ppbc.tile([PD, Bc], dt.float32, tag="rsbc", name="rsbc")
                    nc.tensor.matmul(rs_bc, ones_row_b, rstd_h, start=True, stop=True)
                    rs_b = plb.tile([PD, Bc], dt.bfloat16, tag="rs_b", name="rs_b")
                    nc.scalar.activation(rs_b, rs_bc, AF.Copy)

                # ------- merged fusion Linear2 + per-pair Linear + LN2 ----
                # m=0's accumulation is interleaved into the LayerNorm loop
                # (each k-matmul paced on gelu[k]) so the PE stays active
                # through the barrier and the HAM clock gate stays warm.
                with (
                    tc.tile_pool(name="wcs", bufs=3) as pwc,
                    tc.tile_pool(name="psM2", bufs=2, space="PSUM") as ppm2,
                    tc.tile_pool(name="tnorm", bufs=2) as ptn,
                    tc.tile_pool(name="hpo", bufs=3) as php,
                    tc.tile_pool(name="rws", bufs=3) as prw,
                    tc.tile_pool(name="yout", bufs=3) as pyo,
                ):
                    hpo_t = [None] * P
                    rows_t = [None] * P
                    wc_t = {}

                    def mm_head(m):
                        wc = pwc.tile([PD, P, PD], dt.bfloat16, tag="wc", name="wc")
                        (nc.sync if m % 2 == 0 else nc.scalar).dma_start(wc, wcr[m])
                        wc_t[m] = wc
                        pm2 = ppm2.tile([PD, Bc], dt.float32, tag="pm2", name="pm2")
                        nc.tensor.matmul(
                            pm2, pw0_sb[:, m, :], psT_sb[m], start=True, stop=False,
                            skip_group_check=True,
                        )
                        return pm2

                    def mm_k(pm2, m, k):
                        nc.tensor.matmul(
                            pm2, wc_t[m][:, k, :], hbuf[k],
                            start=False, stop=(k == P - 1 and not has_c),
                            skip_group_check=True,
                        )

                    def mm_tail(pm2, m):
                        if has_c:
                            nc.tensor.matmul(
                                pm2, c_sb[:, m, :], ones_bc, start=False, stop=True,
                                skip_group_check=True,
                            )
                        del wc_t[m]
                        hpo = php.tile([PD, Bc], dt.bfloat16, tag="hpo", name="hpo")
                        nc.scalar.activation(hpo, pm2, AF.Copy)
                        sq2 = php.tile([PD, Bc], dt.bfloat16, tag="sq2", name="sq2")
                        nc.vector.scalar_tensor_tensor(
                            sq2, hpo, 1.0, hpo, op0=ALU.mult, op1=ALU.mult,
                        )
                        hpo_t[m] = (hpo, sq2)

                    pm2_0 = mm_head(0)
                    for n in range(P):
                        t2 = ptn.tile([PD, Bc], dt.bfloat16, tag="t2", name="t2")
                        nc.vector.scalar_tensor_tensor(
                            t2, t1buf[n], 1.0, rs_b,
                            op0=ALU.mult, op1=ALU.mult,
                        )
                        # gelu(t2 * g + b): gain rides the activation scale
                        nc.scalar.activation(
                            hbuf[n], t2, AF.Gelu,
                            bias=lnc["be1"][:, n:n + 1],
                            scale=lnc["g1"][:, n:n + 1],
                        )
                        mm_k(pm2_0, 0, n)
                        dummy_mm()
                        dummy_mm()
                    mm_tail(pm2_0, 0)

                    with (
                        tc.tile_pool(name="st2", bufs=1, space="PSUM") as pst2,
                        tc.tile_pool(name="bc2", bufs=1, space="PSUM") as pbc2,
                    ):
                        st2h = pst2.tile([1, Bc], dt.float32, tag="st2h", name="st2h")
                        st2q = pst2.tile([1, Bc], dt.float32, tag="st2q", name="st2q")

                        def emit_stats(m):
                            hpo, sq2 = hpo_t[m]
                            nc.tensor.matmul(
                                st2h, ones_col, hpo, start=True, stop=True,
                                skip_group_check=True,
                            )
                            nc.tensor.matmul(
                                st2q, ones_col, sq2, start=True, stop=True,
                                skip_group_check=True,
                            )
                            mu2 = prw.tile([1, Bc], dt.bfloat16, tag="mu2", name="mu2")
                            nc.scalar.activation(mu2, st2h, AF.Copy, scale=1.0 / PD)
                            m22 = prw.tile([1, Bc], dt.float32, tag="m22", name="m22")
                            nc.scalar.activation(m22, st2q, AF.Copy, scale=1.0 / PD)
                            v2 = prw.tile([1, Bc], dt.float32, tag="v2", name="v2")
                            nc.vector.scalar_tensor_tensor(
                                v2, mu2, 1.0, mu2, op0=ALU.mult, op1=ALU.mult,
                            )
                            nc.vector.scalar_tensor_tensor(
                                v2, v2, 1.0, m22, op0=ALU.mult, op1=ALU.subtract,
                            )
                            sd2 = prw.tile([1, Bc], dt.float32, tag="sd2", name="sd2")
                            nc.scalar.activation(sd2, v2, AF.Sqrt, bias=eps1, scale=-1.0)
                            rs2 = prw.tile([1, Bc], dt.float32, tag="rs2", name="rs2")
                            nc.vector.reciprocal_approx_fast(rs2, sd2)
                            rs2h = prw.tile([1, Bc], dt.bfloat16, tag="rs2h", name="rs2h")
                            nc.scalar.activation(rs2h, rs2, AF.Copy)
                            rows_t[m] = (mu2, rs2h)

                        def emit_norm(m):
                            mu2, rs2h = rows_t[m]
                            mu2bc = pbc2.tile([PD, Bc], dt.float32, tag="mu2bc", name="mu2bc")
                            nc.tensor.matmul(mu2bc, ones_row_b, mu2, start=True, stop=True,
                                             skip_group_check=True)
                            rs2bc = pbc2.tile([PD, Bc], dt.float32, tag="rs2bc", name="rs2bc")
                            nc.tensor.matmul(rs2bc, ones_row_b, rs2h, start=True, stop=True,
                                             skip_group_check=True)
                            mu2b = pyo.tile([PD, Bc], dt.bfloat16, tag="mu2b", name="mu2b")
                            nc.scalar.activation(mu2b, mu2bc, AF.Copy)
                            rs2b = pyo.tile([PD, Bc], dt.bfloat16, tag="rs2b", name="rs2b")
                            nc.scalar.activation(rs2b, rs2bc, AF.Copy)
                            hpo, _ = hpo_t[m]
                            t1c = pyo.tile([PD, Bc], dt.bfloat16, tag="t1c", name="t1c")
                            nc.vector.scalar_tensor_tensor(
                                t1c, hpo, 1.0, mu2b, op0=ALU.mult, op1=ALU.subtract,
                            )
                            y = pyo.tile([PD, Bc], dt.bfloat16, tag="y", name="y")
                            nc.vector.scalar_tensor_tensor(
                                y, t1c, 1.0, rs2b, op0=ALU.mult, op1=ALU.mult,
                            )
                            if has_g2 or has_bl2:
                                y2 = pyo.tile([PD, Bc], dt.bfloat16, tag="y2", name="y2")
                                nc.scalar.activation(
                                    y2, y, AF.Identity,
                                    bias=lnc["bl2"][:, m:m + 1],
                                    scale=lnc["g2"][:, m:m + 1],
                                )
                                y = y2
                            nc.gpsimd.dma_start(out[m], y)

                        # lag-2 schedule: PE never waits on the
                        # scalar/vector chain
                        for m in range(1, P):
                            pm2 = mm_head(m)
                            for k in range(P):
                                mm_k(pm2, m, k)
                            mm_tail(pm2, m)
                            emit_stats(m - 1)
                            if m > 1:
                                emit_norm(m - 2)
                        for _ in range(14):
                            dummy_mm()
                        emit_stats(P - 1)
                        for _ in range(14):
                            dummy_mm()
                        emit_norm(P - 2)
                        for _ in range(14):
                            dummy_mm()
                        emit_norm(P - 1)

    nc.compile()
    return nc


_CACHE = {}


def _get_nc(has_c, has_bl2, has_g2):
    key = (has_c, has_bl2, has_g2)
    if key not in _CACHE:
        _CACHE[key] = _build(has_c, has_bl2, has_g2)
    return _CACHE[key]


def _prep(inputs):
    f32 = np.float32
    g = lambda k: np.asarray(inputs[k], f32)

    psT_full = np.asarray(g("pair_states").transpose(1, 2, 0), dtype=BF)   # [P,PD,B]
    msT_full = np.asarray(g("macro_state").T, dtype=BF)                    # [MD,B]

    w1 = g("fusion_w1")
    w2 = g("fusion_w2")
    pw = g("pair_w")
    b2 = g("fusion_b2")
    pb = g("pair_b")

    # fold fusion_b1 through the softmax (rows sum to 1)
    vwp = (g("mem_pair_vals") @ w1[:D] + g("fusion_b1")[None, :]).astype(BF)
    vwm = (g("mem_macro_vals") @ w1[D:]).astype(BF)                # (S, D)
    # Wc[m] = W2[:, m-block] @ pw1[m]  -> [m, kp, kt, e] tiling
    w2b = w2.reshape(D, P, PD)
    wc = np.einsum("dpk,pke->pde", w2b, pw[:, PD:, :])             # (P, D, PD)
    wcr = np.ascontiguousarray(
        wc.reshape(P, P, PD, PD).transpose(0, 2, 1, 3)
    ).astype(BF)                                                   # [m, kp, kt, e]
    cvec = np.einsum("pk,pke->pe", b2.reshape(P, PD), pw[:, PD:, :]) + pb
    has_c = bool(np.abs(cvec).max() > 0)
    bl2 = g("pair_ln_b")
    has_bl2 = bool(np.abs(bl2).max() > 0)
    has_g2 = bool(np.abs(g("pair_ln_g") - 1.0).max() > 0)
    vws = np.stack([
        g("mem_pair_vals") @ (w1[:D].sum(1) + g("fusion_b1").sum()),
        g("mem_macro_vals") @ w1[D:].sum(1),
    ], axis=1).astype(BF)                                          # (S, 2)

    shared = {
        "kP": np.ascontiguousarray(
            (g("mem_pair_keys").T / (P * np.sqrt(PD))).astype(BF)),
        "kM": np.ascontiguousarray(
            (g("mem_macro_keys").T / np.sqrt(MD)).astype(BF)),
        "vwp": vwp,
        "vwm": vwm,
        "vws": vws,
        "b1t": np.ascontiguousarray(g("fusion_b1").reshape(P, PD).T),
        "g1t": np.ascontiguousarray(g("fusion_ln_g").reshape(P, PD).T),
        "be1t": np.ascontiguousarray(g("fusion_ln_b").reshape(P, PD).T),
        "wcr": wcr,
        "pw0r": np.ascontiguousarray(
            pw[:, :PD, :].transpose(1, 0, 2)).astype(BF),          # [d, m, e]
        "cbr": np.ascontiguousarray(cvec[None]).astype(BF),        # [1, m, e]
        "g2t": np.ascontiguousarray(g("pair_ln_g").T),             # [e, m]
        "bl2t": np.ascontiguousarray(bl2.T),
    }
    in_maps = []
    for c in range(NCORES):
        m = dict(shared)
        m["psT"] = np.ascontiguousarray(psT_full[:, :, c * Bc:(c + 1) * Bc])
        m["msT"] = np.ascontiguousarray(msT_full[:, c * Bc:(c + 1) * Bc])
        in_maps.append(m)
    return in_maps, (has_c, has_bl2, has_g2)


def _run(inputs, trace=False):
    in_maps, flags = _prep(inputs)
    nc = _get_nc(*flags)
    res = bass_utils.run_bass_kernel_spmd(
        nc, in_maps, core_ids=list(range(NCORES)), trace=trace
    )
    # out [P, PD, Bc] (feature-major) -> (Bc, P, PD) per core
    outp = np.concatenate(
        [
            np.asarray(res.results[c]["out"], np.float32).transpose(2, 0, 1)
            for c in range(NCORES)
        ],
        axis=0,
    )
    return np.ascontiguousarray(outp), res


def kernel(**inputs):
    outp, _ = _run(inputs, trace=False)
    return outp


# revision 11
# speedup vs baseline: 1.0152x; 1.0152x over previous
"""Trainium2 Bass kernel for nn_CrossPairMemory.

Sharding: data-parallel over batch across 8 NeuronCores (512 rows each).

Key algebraic restructuring (host-side, exact):
  h_pre = concat(pair_corr, macro_corr) @ w1
        = attn_p @ (vP @ W1p) + attn_m @ (vM @ W1m)
  so the 7168-deep fusion contraction collapses to two 64-deep matmuls
  against precomputed [64, 3584] tables.  Likewise the per-pair output
  path folds w2 into pair_w:
  out_pre^T[m] = pw0[m]^T ps[m]^T + sum_k Wc[m,k]^T h2[k] + c[m] 1^T
  with Wc[m] = W2[:, m-block] @ pw1[m] and c[m] = b2[m-block] @ pw1[m]
  + pair_b[m], merging the second fusion Linear and the per-pair Linear
  into one accumulation per pair with batch on the free axis.  Both
  LayerNorms are column-stat normalizations via ones-matmuls + rank-1
  broadcasts on the PE; gains are per-partition stt scalars.

Engine balance: row math on GPSIMD, normalize stt alternates
vector/GPSIMD, copies on scalar, reciprocal via the fast custom-DVE
approximation.  Dummy matmuls keep the PE HAM clock-gate warm through
the DMA-bound head and the LayerNorm barrier.
"""

import sys

for _p in ("/opt/trn_rl_repo",):
    if _p not in sys.path:
        sys.path.insert(0, _p)

import numpy as np
import ml_dtypes

import concourse.bass as bass
import concourse.tile as tile
from concourse import bacc, mybir
from concourse import bass_utils

BF = ml_dtypes.bfloat16
dt = mybir.dt
AF = mybir.ActivationFunctionType
ALU = mybir.AluOpType

NCORES = 8
B, P, PD, MD, S = 4096, 28, 128, 256, 64
D = P * PD            # 3584
Bc = B // NCORES      # 512 batch rows per core
EPS = 1e-5


def _build(has_c, has_bl2, has_g2):
    nc = bacc.Bacc(
        "TRN2", target_bir_lowering=False, debug=False, num_devices=NCORES
    )

    def din(name, shape, dty):
        return nc.dram_tensor(name, list(shape), dty, kind="ExternalInput").ap()

    psT = din("psT", (P, PD, Bc), dt.bfloat16)      # pair_states^T per pair
    msT = din("msT", (MD, Bc), dt.bfloat16)         # macro_state^T
    kP = din("kP", (PD, S), dt.bfloat16)            # pair keys^T, pre-scaled
    kM = din("kM", (MD, S), dt.bfloat16)            # macro keys^T, pre-scaled
    vwp = din("vwp", (S, D), dt.bfloat16)           # vP @ W1p
    vwm = din("vwm", (S, D), dt.bfloat16)           # vM @ W1m
    vws = din("vws", (S, 2), dt.bfloat16)           # rowsums for stat_h fold
    b1t = din("b1t", (PD, P), dt.float32)
    g1t = din("g1t", (PD, P), dt.float32)
    be1t = din("be1t", (PD, P), dt.float32)
    wcr = din("wcr", (P, PD, P, PD), dt.bfloat16)   # [m, kp, kt, e]
    pw0r = din("pw0r", (PD, P, PD), dt.bfloat16)    # [d, m, e]
    cbr = din("cbr", (1, P, PD), dt.bfloat16)       # folded bias rows
    g2t = din("g2t", (PD, P), dt.float32)          # pair_ln_g^T [e, m]
    bl2t = din("bl2t", (PD, P), dt.float32)         # pair_ln_b^T [e, m]
    out = nc.dram_tensor(
        "out", [P, PD, Bc], dt.bfloat16, kind="ExternalOutput"
    ).ap()

    with tile.TileContext(nc) as tc:
        with (
            tc.tile_pool(name="const", bufs=1) as const,
            tc.tile_pool(name="res", bufs=1) as res,
            tc.tile_pool(name="psJ", bufs=1, space="PSUM") as psj,
        ):
            # -------- constants + high-priority DMAs (macro path first) ---
            kM0 = const.tile([PD, S], dt.bfloat16, tag="kM0", name="kM0")
            nc.sync.dma_start(kM0, kM[0:PD])
            kM1 = const.tile([PD, S], dt.bfloat16, tag="kM1", name="kM1")
            nc.sync.dma_start(kM1, kM[PD:MD])
            ms0 = const.tile([PD, Bc], dt.bfloat16, tag="ms0", name="ms0")
            nc.sync.dma_start(ms0, msT[0:PD])
            ms1 = const.tile([PD, Bc], dt.bfloat16, tag="ms1", name="ms1")
            nc.sync.dma_start(ms1, msT[PD:MD])
            kP_sb = const.tile([PD, S], dt.bfloat16, tag="kP", name="kP")
            nc.sync.dma_start(kP_sb, kP)

            G = P // 4
            psT_q = []
            for i in range(4):
                t = res.tile([PD, G, Bc], dt.bfloat16, tag=f"psTq{i}", name=f"psTq{i}")
                eng = nc.sync if i % 2 == 0 else nc.scalar
                eng.dma_start(
                    t,
                    bass.AP(tensor=psT.tensor, offset=psT.offset + i * G * PD * Bc,
                            ap=[[Bc, PD], [PD * Bc, G], [1, Bc]]),
                )
                psT_q.append(t)
            psT_sb = [psT_q[p // G][:, p % G, :] for p in range(P)]

            vwp_sb = res.tile([S, D], dt.bfloat16, tag="vwp", name="vwp")
            nc.scalar.dma_start(vwp_sb, vwp)
            vwm_sb = res.tile([S, D], dt.bfloat16, tag="vwm", name="vwm")
            nc.scalar.dma_start(vwm_sb, vwm)
            vws_sb = const.tile([S, 2], dt.bfloat16, tag="vws", name="vws")
            nc.scalar.dma_start(vws_sb, vws)

            lnc = {}
            for nm, src, dty in (("b1", b1t, dt.float32), ("g1", g1t, dt.float32),
                                 ("be1", be1t, dt.float32), ("g2", g2t, dt.float32),
                                 ("bl2", bl2t, dt.float32)):
                t = const.tile([PD, P], dty, tag=f"lnc_{nm}", name=f"lnc_{nm}")
                nc.sync.dma_start(t, src)
                lnc[nm] = t
            pw0_sb = const.tile([PD, P, PD], dt.bfloat16, tag="pw0", name="pw0")
            nc.sync.dma_start(pw0_sb, pw0r)
            c_sb = const.tile([1, P, PD], dt.bfloat16, tag="cb", name="cb")
            if has_c:
                nc.sync.dma_start(c_sb, cbr)

            ones_col = const.tile([PD, 1], dt.bfloat16, tag="ones_col", name="ones_col")
            nc.vector.memset(ones_col, 1.0)
            ones_row_b = const.tile([1, PD], dt.bfloat16, tag="ones_row_b", name="ones_row_b")
            nc.vector.memset(ones_row_b, 1.0)
            ones_bc = const.tile([1, Bc], dt.bfloat16, tag="ones_bc", name="ones_bc")
            nc.vector.memset(ones_bc, 1.0)
            eps1 = const.tile([1, 1], dt.float32, tag="eps1", name="eps1")
            nc.vector.memset(eps1, EPS)

            # junk psum target for PE HAM-warming dummy matmuls
            junk = psj.tile([S, Bc], dt.float32, tag="junk", name="junk")

            def dummy_mm():
                nc.tensor.matmul(junk, kP_sb, ms0, start=True, stop=True,
                                 skip_group_check=True)

            hbuf = [
                res.tile([PD, Bc], dt.bfloat16, tag=f"hb{n}", name=f"hb{n}")
                for n in range(P)
            ]
            t1buf = [
                res.tile([PD, Bc], dt.bfloat16, tag=f"t1b{n}", name=f"t1b{n}")
                for n in range(P)
            ]
            ab = {}

            # ---------------- stage A: attention weights ----------------
            with (
                tc.tile_pool(name="stA", bufs=1) as pa,
                tc.tile_pool(name="psA", bufs=2, space="PSUM") as ppa,
            ):
                def softmax_read(which):
                    sp = ppa.tile([S, Bc], dt.float32, tag="sp", name="sp")
                    if which == "pair":
                        for p in range(P):
                            dummy_mm()
                            if p < 14:
                                dummy_mm()
                            nc.tensor.matmul(
                                sp, kP_sb, psT_sb[p],
                                start=(p == 0), stop=(p == P - 1),
                                skip_group_check=True,
                            )
                    else:
                        nc.tensor.matmul(sp, kM0, ms0, start=True, stop=False)
                        nc.tensor.matmul(sp, kM1, ms1, start=False, stop=True)
                    # scores are O(0.3): exp without max-subtraction is safe
                    eb = pa.tile([S, Bc], dt.bfloat16, tag=f"eb_{which}", name=f"eb_{which}")
                    nc.scalar.activation(eb, sp, AF.Exp)
                    den = ppa.tile([1, Bc], dt.float32, tag="den", name="den")
                    nc.tensor.matmul(den, ones_col[0:S, :], eb, start=True, stop=True,
                                     skip_group_check=True)
                    rr = pa.tile([1, Bc], dt.float32, tag=f"rr_{which}", name=f"rr_{which}")
                    nc.vector.reciprocal_approx_fast(rr, den)
                    rrh = pa.tile([1, Bc], dt.bfloat16, tag=f"rrh_{which}", name=f"rrh_{which}")
                    nc.scalar.activation(rrh, rr, AF.Copy)
                    rbc = ppa.tile([S, Bc], dt.float32, tag="rbc", name="rbc")
                    nc.tensor.matmul(
                        rbc, ones_row_b[:, 0:S], rrh, start=True, stop=True,
                        skip_group_check=True,
                    )
                    t = res.tile([S, Bc], dt.bfloat16, tag=f"ab_{which}", name=f"ab_{which}")
                    nc.vector.tensor_mul(t, eb, rbc)
                    ab[which] = t

                softmax_read("macro")
                softmax_read("pair")

            # ------------- stage B: folded fusion Linear1 + stats --------
            with tc.tile_pool(name="stat", bufs=1, space="PSUM") as pst:
                stat_h = pst.tile([1, Bc], dt.float32, tag="stat_h", name="stat_h")
                stat_q = pst.tile([1, Bc], dt.float32, tag="stat_q", name="stat_q")

                with (
                    tc.tile_pool(name="psB", bufs=2, space="PSUM") as ppm,
                    tc.tile_pool(name="sqp", bufs=2) as psq,
                ):
                    sq_t = [None] * P

                    # stat_h folds through the linear algebra: softmax rows
                    # sum to 1, so sum_f h = ab_p @ rowsum(VWp + b1) +
                    # ab_m @ rowsum(VWm)  (host-folded into vws)
                    nc.tensor.matmul(
                        stat_h, vws_sb[:, 0:1], ab["pair"],
                        start=True, stop=False, skip_group_check=True,
                    )
                    nc.tensor.matmul(
                        stat_h, vws_sb[:, 1:2], ab["macro"],
                        start=False, stop=True, skip_group_check=True,
                    )
                    # mean side of LayerNorm1 is ready now — broadcast it
                    # early so stage B can also fold in the mean-subtract
                    mu_h = const.tile([1, Bc], dt.bfloat16, tag="muh", name="muh")
                    nc.scalar.activation(mu_h, stat_h, AF.Copy, scale=1.0 / D)
                    mu_bc0 = ppm.tile([PD, Bc], dt.float32, tag="pm", name="pm")
                    nc.tensor.matmul(mu_bc0, ones_row_b, mu_h, start=True, stop=True,
                                     skip_group_check=True)
                    mu_b = const.tile([PD, Bc], dt.bfloat16, tag="mu_b", name="mu_b")
                    nc.scalar.activation(mu_b, mu_bc0, AF.Copy)

                    def stats_for(n):
                        nc.tensor.matmul(
                            stat_q, ones_col, sq_t[n],
                            start=(n == 0), stop=(n == P - 1),
                            skip_group_check=True,
                        )

                    for n in range(P):
                        pm = ppm.tile([PD, Bc], dt.float32, tag="pm", name="pm")
                        ns = slice(n * PD, (n + 1) * PD)
                        nc.tensor.matmul(
                            pm, vwp_sb[:, ns], ab["pair"], start=True, stop=False
                        )
                        nc.tensor.matmul(
                            pm, vwm_sb[:, ns], ab["macro"], start=False, stop=True
                        )
                        # lag the stats matmuls two iterations so the PE
                        # never waits on the copy/square of the current tile
                        if n > 1:
                            stats_for(n - 2)
                        nc.scalar.activation(hbuf[n], pm, AF.Copy)
                        sq = psq.tile([PD, Bc], dt.bfloat16, tag="sq", name="sq")
                        nc.vector.scalar_tensor_tensor(
                            sq, hbuf[n], 1.0, hbuf[n],
                            op0=ALU.mult, op1=ALU.mult,
                        )
                        nc.vector.scalar_tensor_tensor(
                            t1buf[n], hbuf[n], 1.0, mu_b,
                            op0=ALU.mult, op1=ALU.subtract,
                        )
                        sq_t[n] = sq
                    stats_for(P - 2)
                    stats_for(P - 1)
                    for _ in range(16):
                        dummy_mm()

                # ---------------- LayerNorm1 + gelu ----------------------
                with tc.tile_pool(name="lnrow", bufs=1) as plr:
                    m2_row = plr.tile([1, Bc], dt.float32, tag="m2", name="m2")
                    nc.scalar.activation(m2_row, stat_q, AF.Copy, scale=1.0 / D)
                    nva = plr.tile([1, Bc], dt.float32, tag="va", name="va")
                    nc.vector.scalar_tensor_tensor(
                        nva, mu_h, 1.0, mu_h, op0=ALU.mult, op1=ALU.mult,
                    )
                    nc.vector.scalar_tensor_tensor(
                        nva, nva, 1.0, m2_row, op0=ALU.mult, op1=ALU.subtract,
                    )
                    # nva = mu^2 - m2 = -var; sqrt(nva * -1 + eps) = sd
                    sd_row = plr.tile([1, Bc], dt.float32, tag="sd", name="sd")
                    nc.scalar.activation(
                        sd_row, nva, AF.Sqrt, bias=eps1, scale=-1.0
                    )
                    rstd_row = plr.tile([1, Bc], dt.float32, tag="rs", name="rs")
                    nc.vector.reciprocal_approx_fast(rstd_row, sd_row)
                    rstd_h = plr.tile([1, Bc], dt.bfloat16, tag="rsh", name="rsh")
                    nc.scalar.activation(rstd_h, rstd_row, AF.Copy)

            # stat psum closed; broadcast the rstd row
            with tc.tile_pool(name="lnb", bufs=1) as plb:
                with tc.tile_pool(name="psBC", bufs=1, space="PSUM") as ppbc:
                    rs_bc = ppbc.tile([PD, Bc], dt.float32, tag="rsbc", name="rsbc")
                    nc.tensor.matmul(rs_bc, ones_row_b, rstd_h, start=True, stop=True)
                    rs_b = plb.tile([PD, Bc], dt.bfloat16, tag="rs_b", name="rs_b")
                    nc.scalar.activation(rs_b, rs_bc, AF.Copy)

                # ------- merged fusion Linear2 + per-pair Linear + LN2 ----
                # m=0's accumulation is interleaved into the LayerNorm loop
                # (each k-matmul paced on gelu[k]) so the PE stays active
                # through the barrier and the HAM clock gate stays warm.
                with (
                    tc.tile_pool(name="wcs", bufs=3) as pwc,
                    tc.tile_pool(name="psM2", bufs=2, space="PSUM") as ppm2,
                    tc.tile_pool(name="tnorm", bufs=2) as ptn,
                    tc.tile_pool(name="hpo", bufs=3) as php,
                    tc.tile_pool(name="rws", bufs=3) as prw,
                    tc.tile_pool(name="yout", bufs=3) as pyo,
                ):
                    hpo_t = [None] * P
                    rows_t = [None] * P
                    wc_t = {}

                    def mm_head(m):
                        wc = pwc.tile([PD, P, PD], dt.bfloat16, tag="wc", name="wc")
                        nc.sync.dma_start(wc, wcr[m])
                        wc_t[m] = wc
                        pm2 = ppm2.tile([PD, Bc], dt.float32, tag="pm2", name="pm2")
                        nc.tensor.matmul(
                            pm2, pw0_sb[:, m, :], psT_sb[m], start=True, stop=False,
                            skip_group_check=True,
                        )
                        return pm2

                    def mm_k(pm2, m, k):
                        nc.tensor.matmul(
                            pm2, wc_t[m][:, k, :], hbuf[k],
                            start=False, stop=(k == P - 1 and not has_c),
                            skip_group_check=True,
                        )

                    def mm_tail(pm2, m):
                        if has_c:
                            nc.tensor.matmul(
                                pm2, c_sb[:, m, :], ones_bc, start=False, stop=True,
                                skip_group_check=True,
                            )
                        del wc_t[m]
                        hpo = php.tile([PD, Bc], dt.bfloat16, tag="hpo", name="hpo")
                        nc.scalar.activation(hpo, pm2, AF.Copy)
                        sq2 = php.tile([PD, Bc], dt.bfloat16, tag="sq2", name="sq2")
                        nc.vector.scalar_tensor_tensor(
                            sq2, hpo, 1.0, hpo, op0=ALU.mult, op1=ALU.mult,
                        )
                        hpo_t[m] = (hpo, sq2)

                    pm2_0 = mm_head(0)
                    for n in range(P):
                        t2 = ptn.tile([PD, Bc], dt.bfloat16, tag="t2", name="t2")
                        nc.vector.scalar_tensor_tensor(
                            t2, t1buf[n], 1.0, rs_b,
                            op0=ALU.mult, op1=ALU.mult,
                        )
                        # gelu(t2 * g + b): gain rides the activation scale
                        nc.scalar.activation(
                            hbuf[n], t2, AF.Gelu,
                            bias=lnc["be1"][:, n:n + 1],
                            scale=lnc["g1"][:, n:n + 1],
                        )
                        mm_k(pm2_0, 0, n)
                        dummy_mm()
                        dummy_mm()
                    mm_tail(pm2_0, 0)

                    with (
                        tc.tile_pool(name="st2", bufs=1, space="PSUM") as pst2,
                        tc.tile_pool(name="bc2", bufs=1, space="PSUM") as pbc2,
                    ):
                        st2h = pst2.tile([1, Bc], dt.float32, tag="st2h", name="st2h")
                        st2q = pst2.tile([1, Bc], dt.float32, tag="st2q", name="st2q")

                        def emit_stats(m):
                            hpo, sq2 = hpo_t[m]
                            nc.tensor.matmul(
                                st2h, ones_col, hpo, start=True, stop=True,
                                skip_group_check=True,
                            )
                            nc.tensor.matmul(
                                st2q, ones_col, sq2, start=True, stop=True,
                                skip_group_check=True,
                            )
                            mu2 = prw.tile([1, Bc], dt.bfloat16, tag="mu2", name="mu2")
                            nc.scalar.activation(mu2, st2h, AF.Copy, scale=1.0 / PD)
                            m22 = prw.tile([1, Bc], dt.float32, tag="m22", name="m22")
                            nc.scalar.activation(m22, st2q, AF.Copy, scale=1.0 / PD)
                            v2 = prw.tile([1, Bc], dt.float32, tag="v2", name="v2")
                            nc.vector.scalar_tensor_tensor(
                                v2, mu2, 1.0, mu2, op0=ALU.mult, op1=ALU.mult,
                            )
                            nc.vector.scalar_tensor_tensor(
                                v2, v2, 1.0, m22, op0=ALU.mult, op1=ALU.subtract,
                            )
                            sd2 = prw.tile([1, Bc], dt.float32, tag="sd2", name="sd2")
                            nc.scalar.activation(sd2, v2, AF.Sqrt, bias=eps1, scale=-1.0)
                            rs2 = prw.tile([1, Bc], dt.float32, tag="rs2", name="rs2")
                            nc.vector.reciprocal_approx_fast(rs2, sd2)
                            rs2h = prw.tile([1, Bc], dt.bfloat16, tag="rs2h", name="rs2h")
                            nc.scalar.activation(rs2h, rs2, AF.Copy)
                            rows_t[m] = (mu2, rs2h)

                        def emit_norm(m):
                            mu2, rs2h = rows_t[m]
                            mu2bc = pbc2.tile([PD, Bc], dt.float32, tag="mu2bc", name="mu2bc")
                            nc.tensor.matmul(mu2bc, ones_row_b, mu2, start=True, stop=True,
                                             skip_group_check=True)
                            rs2bc = pbc2.tile([PD, Bc], dt.float32, tag="rs2bc", name="rs2bc")
                            nc.tensor.matmul(rs2bc, ones_row_b, rs2h, start=True, stop=True,
                                             skip_group_check=True)
                            mu2b = pyo.tile([PD, Bc], dt.bfloat16, tag="mu2b", name="mu2b")
                            nc.scalar.activation(mu2b, mu2bc, AF.Copy)
                            rs2b = pyo.tile([PD, Bc], dt.bfloat16, tag="rs2b", name="rs2b")
                            nc.scalar.activation(rs2b, rs2bc, AF.Copy)
                            hpo, _ = hpo_t[m]
                            t1c = pyo.tile([PD, Bc], dt.bfloat16, tag="t1c", name="t1c")
                            nc.vector.scalar_tensor_tensor(
                                t1c, hpo, 1.0, mu2b, op0=ALU.mult, op1=ALU.subtract,
                            )
                            y = pyo.tile([PD, Bc], dt.bfloat16, tag="y", name="y")
                            nc.vector.scalar_tensor_tensor(
                                y, t1c, 1.0, rs2b, op0=ALU.mult, op1=ALU.mult,
                            )
                            if has_g2 or has_bl2:
                                y2 = pyo.tile([PD, Bc], dt.bfloat16, tag="y2", name="y2")
                                nc.scalar.activation(
                                    y2, y, AF.Identity,
                                    bias=lnc["bl2"][:, m:m + 1],
                                    scale=lnc["g2"][:, m:m + 1],
                                )
                                y = y2
                            nc.gpsimd.dma_start(out[m], y)

                        # lag-2 schedule: PE never waits on the
                        # scalar/vector chain
                        for m in range(1, P):
                            pm2 = mm_head(m)
                            for k in range(P):
                                mm_k(pm2, m, k)
                            mm_tail(pm2, m)
                            emit_stats(m - 1)
                            if m > 1:
                                emit_norm(m - 2)
                        for _ in range(14):
                            dummy_mm()
                        emit_stats(P - 1)
                        for _ in range(14):
                            dummy_mm()
                        emit_norm(P - 2)
                        for _ in range(14):
                            dummy_mm()
                        emit_norm(P - 1)

    nc.compile()
    return nc


_CACHE = {}


def _get_nc(has_c, has_bl2, has_g2):
    key = (has_c, has_bl2, has_g2)
    if key not in _CACHE:
        _CACHE[key] = _build(has_c, has_bl2, has_g2)
    return _CACHE[key]


def _prep(inputs):
    f32 = np.float32
    g = lambda k: np.asarray(inputs[k], f32)

    psT_full = np.asarray(g("pair_states").transpose(1, 2, 0), dtype=BF)   # [P,PD,B]
    msT_full = np.asarray(g("macro_state").T, dtype=BF)                    # [MD,B]

    w1 = g("fusion_w1")
    w2 = g("fusion_w2")
    pw = g("pair_w")
    b2 = g("fusion_b2")
    pb = g("pair_b")

    # fold fusion_b1 through the softmax (rows sum to 1)
    vwp = (g("mem_pair_vals") @ w1[:D] + g("fusion_b1")[None, :]).astype(BF)
    vwm = (g("mem_macro_vals") @ w1[D:]).astype(BF)                # (S, D)
    # Wc[m] = W2[:, m-block] @ pw1[m]  -> [m, kp, kt, e] tiling
    w2b = w2.reshape(D, P, PD)
    wc = np.einsum("dpk,pke->pde", w2b, pw[:, PD:, :])             # (P, D, PD)
    wcr = np.ascontiguousarray(
        wc.reshape(P, P, PD, PD).transpose(0, 2, 1, 3)
    ).astype(BF)                                                   # [m, kp, kt, e]
    cvec = np.einsum("pk,pke->pe", b2.reshape(P, PD), pw[:, PD:, :]) + pb
    has_c = bool(np.abs(cvec).max() > 0)
    bl2 = g("pair_ln_b")
    has_bl2 = bool(np.abs(bl2).max() > 0)
    has_g2 = bool(np.abs(g("pair_ln_g") - 1.0).max() > 0)
    vws = np.stack([
        g("mem_pair_vals") @ (w1[:D].sum(1) + g("fusion_b1").sum()),
        g("mem_macro_vals") @ w1[D:].sum(1),
    ], axis=1).astype(BF)                                          # (S, 2)

    shared = {
        "kP": np.ascontiguousarray(
            (g("mem_pair_keys").T / (P * np.sqrt(PD))).astype(BF)),
        "kM": np.ascontiguousarray(
            (g("mem_macro_keys").T / np.sqrt(MD)).astype(BF)),
        "vwp": vwp,
        "vwm": vwm,
        "vws": vws,
        "b1t": np.ascontiguousarray(g("fusion_b1").reshape(P, PD).T),
        "g1t": np.ascontiguousarray(g("fusion_ln_g").reshape(P, PD).T),
        "be1t": np.ascontiguousarray(g("fusion_ln_b").reshape(P, PD).T),
        "wcr": wcr,
        "pw0r": np.ascontiguousarray(
            pw[:, :PD, :].transpose(1, 0, 2)).astype(BF),          # [d, m, e]
        "cbr": np.ascontiguousarray(cvec[None]).astype(BF),        # [1, m, e]
        "g2t": np.ascontiguousarray(g("pair_ln_g").T),             # [e, m]
        "bl2t": np.ascontiguousarray(bl2.T),
    }
    in_maps = []
    for c in range(NCORES):
        m = dict(shared)
        m["psT"] = np.ascontiguousarray(psT_full[:, :, c * Bc:(c + 1) * Bc])
        m["msT"] = np.ascontiguousarray(msT_full[:, c * Bc:(c + 1) * Bc])
        in_maps.append(m)
    return in_maps, (has_c, has_bl2, has_g2)


def _run(inputs, trace=False):
    in_maps, flags = _prep(inputs)
    nc = _get_nc(*flags)
    res = bass_utils.run_bass_kernel_spmd(
        nc, in_maps, core_ids=list(range(NCORES)), trace=trace
    )
    # out [P, PD, Bc] (feature-major) -> (Bc, P, PD) per core
    outp = np.concatenate(
        [
            np.asarray(res.results[c]["out"], np.float32).transpose(2, 0, 1)
            for c in range(NCORES)
        ],
        axis=0,
    )
    return np.ascontiguousarray(outp), res


def kernel(**inputs):
    outp, _ = _run(inputs, trace=False)
    return outp
